# revision 24
# baseline (speedup 1.0000x reference)
"""Distributed Trainium2 kernel for nn_Attention_29832842838194.

LayerNorm (stats over the *sequence* axis) -> QKV projection -> 8-head
attention (N=2048, d_head=64) -> output projection, on 8 NeuronCores.

Sharding:
  - tokens (B*N = 4096) split 8 ways for LayerNorm / final projection
  - heads (8) split 8 ways for attention (core c owns head c, both batches)
  - collectives: warmup AllReduce (absorbs the one-time CC entry barrier),
    AllReduce of LN stats (16KB), AllGather of raw x^T (bf16), and two
    batch-split AllToAlls of unnormalized per-head attention output (bf16).

Numerics: projections run in bf16; the attention pipeline (q/k/v, exp,
attn@v) runs in fp32 storage with tfloat32 (float32r) matmuls.

The kernel is self-contained: shapes are hardcoded to the problem spec.
"""

import numpy as np

# -------- problem constants (hardcoded per spec) --------
B = 2
NSEQ = 2048  # sequence length per batch
DIM = 1024
HEADS = 8
DHEAD = 64
INNER = HEADS * DHEAD  # 512
EPS = 1e-5
NCORES = 8
P = 128

SCALE = DHEAD ** -0.5  # 0.125


def _cfg(nseq=NSEQ):
    """Derived dims. nseq can be shrunk for simulator tests."""
    T = B * nseq              # total tokens
    TLOC = T // NCORES        # tokens per core
    TB = nseq // NCORES       # tokens per core per batch (a2a shard size)
    assert TLOC % P == 0 and nseq % 512 == 0
    return dict(
        nseq=nseq,
        T=T,
        TLOC=TLOC,
        TB=TB,
        XT=TLOC // P,         # x token-subtiles per core
        DC=DIM // P,          # 8 dmodel chunks
        KC=INNER // P,        # 4 inner chunks
        IB=nseq // 512,       # 512-query i-blocks per batch
        JC=nseq // P,         # 128-key j-chunks per batch
    )


def build_body(tc, outs, ins, cfg):
    """Emit the per-core program. outs/ins are dicts of DRAM APs."""
    import concourse.mybir as mybir
    from concourse.masks import make_identity

    dt = mybir.dt
    AF = mybir.ActivationFunctionType
    ALU = mybir.AluOpType
    nc = tc.nc

    T, TLOC, TB, XT = cfg["T"], cfg["TLOC"], cfg["TB"], cfg["XT"]
    DC, KC, IB, JC = cfg["DC"], cfg["KC"], cfg["IB"], cfg["JC"]
    nseq = cfg["nseq"]
    NTOK = float(nseq)  # tokens per batch (LN normalizer)
    RG = [list(range(NCORES))]

    def f32r(ap):
        return ap.bitcast(dt.float32r)

    x = ins["x"]      # [TLOC, DIM] f32 (this core's token slice)
    g = ins["g"]      # [DIM] f32
    wq = ins["wq"]    # [DIM, DHEAD] f32 (this core's head slice)
    wk = ins["wk"]    # [DIM, DHEAD]
    wv = ins["wv"]    # [DIM, DHEAD]
    wo = ins["wo"]    # [INNER, DIM] f32 (replicated)
    bsel = ins["bsel"]  # [P, 2] f32 one-hot batch selector for this core
    out = outs["out"]  # [TLOC, DIM] f32

    with (
        tc.tile_pool(name="persist", bufs=1) as pp,
        tc.tile_pool(name="loads", bufs=1) as lp,
        tc.tile_pool(name="work", bufs=3) as pool,
        tc.tile_pool(name="work2", bufs=2) as pool2,
        tc.tile_pool(name="psum", bufs=2, space="PSUM") as psum,
        tc.tile_pool(name="dram", bufs=1, space="DRAM") as dram,
    ):
        # -------- phase 0b: loads
        ident = pp.tile([P, P], dt.bfloat16)
        make_identity(nc, ident)
        identf = pp.tile([DHEAD + 1, DHEAD + 1], dt.bfloat16)
        make_identity(nc, identf)

        g_sb = pp.tile([P, DC], dt.float32)
        nc.sync.dma_start(g_sb[:], g.rearrange("(c p) -> p c", p=P))

        bsel_sb = pp.tile([P, 2], dt.float32)
        nc.sync.dma_start(bsel_sb[:], bsel)

        # -------- phase 1: per-tile load -> cast -> transpose; LN stats
        # xT[p, dc, t] = x[t, dc*128+p] (bf16)
        xT = pp.tile([P, DC, TLOC], dt.bfloat16)
        x_bf = pp.tile([P, XT, DIM], dt.bfloat16)
        for t in range(XT):
            x_f = pool.tile([P, DIM], dt.float32, tag="xload")
            nc.sync.dma_start(x_f[:], x[t * P:(t + 1) * P, :])
            nc.vector.tensor_copy(x_bf[:, t, :], x_f[:])
        for dc in range(DC):
            ps = psum.tile([P, TLOC], dt.bfloat16, tag="tr")
            for t in range(XT):
                nc.tensor.transpose(
                    ps[:, t * P:(t + 1) * P],
                    x_bf[:, t, dc * P:(dc + 1) * P],
                    ident[:],
                )
            nc.vector.tensor_copy(xT[:, dc, :], ps[:])

        sums = pp.tile([P, DC], dt.float32)
        sumsq = pp.tile([P, DC], dt.float32)
        nc.vector.tensor_reduce(
            sums[:], xT[:], axis=mybir.AxisListType.X, op=ALU.add
        )
        for dc in range(DC):
            trash = pool2.tile([P, TLOC], dt.float32, tag="trash")
            nc.scalar.activation(
                trash[:], xT[:, dc, :], AF.Square,
                accum_out=sumsq[:, dc:dc + 1],
            )

        # pack [sum_b0 | sum_b1 | sq_b0 | sq_b1] masked by batch one-hot
        stats_sb = pp.tile([P, 4 * DC], dt.float32)
        nc.vector.tensor_scalar_mul(stats_sb[:, 0:DC], sums[:], bsel_sb[:, 0:1])
        nc.vector.tensor_scalar_mul(stats_sb[:, DC:2 * DC], sums[:], bsel_sb[:, 1:2])
        nc.vector.tensor_scalar_mul(stats_sb[:, 2 * DC:3 * DC], sumsq[:], bsel_sb[:, 0:1])
        nc.vector.tensor_scalar_mul(stats_sb[:, 3 * DC:4 * DC], sumsq[:], bsel_sb[:, 1:2])

        # raw x^T AllGather with the packed LN partial stats as a 9th chunk
        ag_in = dram.tile([DC + 1, P, TLOC], dt.bfloat16)
        nc.sync.dma_start(ag_in[0:DC].rearrange("c p t -> p c t"), xT[:])
        nc.sync.dma_start(ag_in[DC, :, 0:8 * DC],
                          stats_sb[:].bitcast(dt.bfloat16))
        ag_out = dram.tile([NCORES, DC + 1, P, TLOC], dt.bfloat16,
                           addr_space="Shared")
        nc.gpsimd.collective_compute(
            "AllGather", ALU.bypass, replica_groups=RG,
            ins=[ag_in.opt()], outs=[ag_out.opt()],
        )
        # sum the 8 cores' partial stats locally
        stats8 = pp.tile([P, NCORES, 4 * DC], dt.float32)
        nc.sync.dma_start(
            stats8[:].bitcast(dt.bfloat16),
            ag_out[:, DC, :, 0:8 * DC].rearrange("r p c -> p r c"),
        )
        stats_f = pp.tile([P, 4 * DC], dt.float32)
        nc.vector.tensor_reduce(
            stats_f[:], stats8[:].rearrange("p r c -> p c r"),
            axis=mybir.AxisListType.X, op=ALU.add,
        )

        # weights (loaded while collectives run)
        w_bf = {}
        for name, wsrc in (("q", wq), ("k", wk), ("v", wv)):
            wl = pool.tile([P, DC, DHEAD], dt.float32, tag="wload")
            nc.sync.dma_start(wl[:], wsrc.rearrange("(c p) m -> p c m", p=P))
            wb = pp.tile([P, DC, DHEAD], dt.bfloat16, tag=f"w{name}")
            nc.vector.tensor_copy(wb[:], wl[:])
            w_bf[name] = wb
        wo_bf = pp.tile([P, KC, DIM], dt.bfloat16)
        for kc in range(KC):
            wol = pool.tile([P, DIM], dt.float32, tag="woload")
            nc.sync.dma_start(wol[:], wo[kc * P:(kc + 1) * P, :])
            nc.vector.tensor_copy(wo_bf[:, kc, :], wol[:])

        # head-broadcast selector matrices for the rownorm broadcast:
        # sel[h, kc, m] = 1 iff h == 2*kc + (m >= DHEAD)
        sel_np = np.zeros((NCORES, KC, P), np.float32)
        for kc in range(KC):
            sel_np[2 * kc, kc, 0:DHEAD] = 1.0
            sel_np[2 * kc + 1, kc, DHEAD:P] = 1.0
        sel_dram = nc.inline_tensor(sel_np, name="selmat")
        sel = pp.tile([NCORES, KC, P], dt.float32)
        nc.sync.dma_start(sel[:], sel_dram.ap())

        # -------- phase 2: LN coefficients
        # mean = sum/NTOK ; var = sq/NTOK - mean^2 ; rstd = sqrt(1/(var+eps))
        mean = pp.tile([P, 2 * DC], dt.float32)
        nc.vector.tensor_scalar_mul(mean[:], stats_f[:, 0:2 * DC], 1.0 / NTOK)
        e2 = pp.tile([P, 2 * DC], dt.float32)
        nc.vector.tensor_scalar_mul(e2[:], stats_f[:, 2 * DC:4 * DC], 1.0 / NTOK)
        msq = pp.tile([P, 2 * DC], dt.float32)
        nc.vector.tensor_tensor(msq[:], mean[:], mean[:], ALU.mult)
        vareps = pp.tile([P, 2 * DC], dt.float32)
        nc.vector.tensor_tensor(vareps[:], e2[:], msq[:], ALU.subtract)
        nc.vector.tensor_scalar_add(vareps[:], vareps[:], EPS)
        rvar = pp.tile([P, 2 * DC], dt.float32)
        nc.vector.reciprocal(rvar[:], vareps[:])
        rstd = pp.tile([P, 2 * DC], dt.float32)
        nc.scalar.activation(rstd[:], rvar[:], AF.Sqrt)

        A2 = pp.tile([P, 2 * DC], dt.float32)
        for b in range(2):
            nc.vector.tensor_tensor(
                A2[:, b * DC:(b + 1) * DC], rstd[:, b * DC:(b + 1) * DC],
                g_sb[:], ALU.mult,
            )
        C2 = pp.tile([P, 2 * DC], dt.float32)
        nc.vector.tensor_tensor(C2[:], mean[:], A2[:], ALU.mult)
        nc.vector.tensor_scalar_mul(C2[:], C2[:], -1.0)

        # -------- phase 3: QKV projections (head slice, all tokens) --------
        qT = pp.tile([DHEAD, T], dt.bfloat16)
        kT = pp.tile([DHEAD, T], dt.bfloat16)
        vT = pp.tile([DHEAD + 1, T], dt.bfloat16)
        nc.vector.memset(vT[DHEAD:DHEAD + 1, :], 1.0)
        dstT = {"q": qT, "k": kT, "v": vT}
        for r in range(NCORES):
            br = (r * TLOC) // nseq  # batch of this token chunk
            xg = pool.tile([P, DC, TLOC], dt.bfloat16, tag="xg")
            nc.sync.dma_start(xg[:], ag_out[r, 0:DC].rearrange("c p t -> p c t"))
            for dc in range(DC):
                nc.vector.tensor_scalar(
                    xg[:, dc, :], xg[:, dc, :],
                    A2[:, br * DC + dc:br * DC + dc + 1],
                    C2[:, br * DC + dc:br * DC + dc + 1],
                    ALU.mult, ALU.add,
                )
            for name in ("q", "k", "v"):
                ps = psum.tile([DHEAD, 512], dt.float32, tag="acc")
                for kc in range(DC):
                    nc.tensor.matmul(
                        ps[:, 0:TLOC], w_bf[name][:, kc, :], xg[:, kc, :],
                        start=(kc == 0), stop=(kc == DC - 1),
                    )
                nc.scalar.copy(
                    dstT[name][0:DHEAD, r * TLOC:(r + 1) * TLOC], ps[:, 0:TLOC]
                )

        # v^T (plus ones row) -> v layout [j, d | 1] for PV + row-norm
        DE = DHEAD + 1
        vext = pp.tile([P, 2 * JC, DE], dt.bfloat16)
        DEP = DE + 1  # pad per-slot stride to keep psum offsets 4B-aligned
        for jg in range(2 * JC // 4):
            ps = psum.tile([P, 4 * DEP], dt.bfloat16, tag="tr")
            for u in range(4):
                jc = jg * 4 + u
                nc.tensor.transpose(
                    ps[:, u * DEP:u * DEP + DE],
                    vT[:, jc * P:(jc + 1) * P],
                    identf[:],
                )
            nc.vector.tensor_copy(
                vext[:, jg * 4:(jg + 1) * 4, :],
                ps[:].rearrange("p (u d) -> p u d", u=4)[:, :, 0:DE],
            )

        # -------- phase 4+5: attention per batch; A2A + out proj per half ----
        aoT = pp.tile([DHEAD + 1, T], dt.bfloat16)
        NH = 2 if IB >= 2 else 1      # a2a rounds per batch
        TBH = TB // NH                # tokens per core per round
        IBH = IB // NH                # i-blocks per round
        for b in range(2):
            for h in range(NH):
                for ibl in range(IBH):
                    ib = h * IBH + ibl
                    i0 = b * nseq + ib * 512
                    otp = psum.tile([DHEAD + 1, 512], dt.float32, tag="acc")
                    for jg in range(JC // 2):
                        sp = psum.tile([P, 1024], dt.float32, tag="sim")
                        for u in range(2):
                            jc = jg * 2 + u
                            j0 = b * nseq + jc * P
                            nc.tensor.matmul(
                                sp[:, u * 512:(u + 1) * 512],
                                kT[:, j0:j0 + P], qT[:, i0:i0 + 512],
                                start=True, stop=True,
                            )
                        et = pool.tile([P, 1024], dt.bfloat16, tag="exp")
                        nc.scalar.activation(et[:], sp[:], AF.Exp, scale=SCALE)
                        for u in range(2):
                            jc = jg * 2 + u
                            nc.tensor.matmul(
                                otp[:], vext[:, b * JC + jc, :],
                                et[:, u * 512:(u + 1) * 512],
                                start=(jg == 0 and u == 0),
                                stop=(jg == JC // 2 - 1 and u == 1),
                            )
                    # unnormalized output + rownorm row -> bf16
                    nc.scalar.copy(aoT[:, i0:i0 + 512], otp[:])

                # ---- A2A this round's tokens; shard s -> core s ----
                off = b * nseq + h * IBH * 512
                a2a_in = dram.tile([NCORES, DHEAD + 1, TBH], dt.bfloat16,
                                   tag=f"a2ai{b}{h}")
                for s_ in range(NCORES):
                    nc.sync.dma_start(
                        a2a_in[s_], aoT[:, off + s_ * TBH:off + (s_ + 1) * TBH]
                    )
                a2a_out = dram.tile([NCORES, DHEAD + 1, TBH], dt.bfloat16,
                                    tag=f"a2ao{b}{h}")
                nc.gpsimd.collective_compute(
                    "AllToAll", ALU.bypass, replica_groups=RG,
                    ins=[a2a_in.opt()], outs=[a2a_out.opt()],
                )

                # ---- normalize + output projection for my TBH tokens --------
                ao_g = pool2.tile([P, KC, TBH], dt.bfloat16, tag="aog")
                for kc in range(KC):
                    for rr in range(P // DHEAD):
                        nc.sync.dma_start(
                            ao_g[rr * DHEAD:(rr + 1) * DHEAD, kc, :],
                            a2a_out[2 * kc + rr, 0:DHEAD, :],
                        )
                rn = pool2.tile([NCORES, TBH], dt.bfloat16, tag="rn")
                nc.sync.dma_start(rn[:], a2a_out[:, DHEAD, :])
                rc = pool2.tile([NCORES, TBH], dt.float32, tag="rc")
                nc.vector.reciprocal(rc[:], rn[:])
                for kc in range(KC):
                    bcp = psum.tile([P, TBH], dt.float32, tag="tr")
                    nc.tensor.matmul(
                        bcp[:], sel[:, kc, :], rc[:],
                        start=True, stop=True,
                    )
                    nc.vector.tensor_tensor(
                        ao_g[:, kc, :], ao_g[:, kc, :], bcp[:], ALU.mult
                    )
                for t0 in range(0, TBH, P):
                    mw = min(P, TBH - t0)
                    out_sb = pool2.tile([P, DIM], dt.float32, tag="osb")
                    for nh2 in range(DIM // 512):
                        op = psum.tile([P, 512], dt.float32, tag="tr")
                        for kc in range(KC):
                            nc.tensor.matmul(
                                op[0:mw, :], ao_g[:, kc, t0:t0 + mw],
                                wo_bf[:, kc, nh2 * 512:(nh2 + 1) * 512],
                                start=(kc == 0), stop=(kc == KC - 1),
                            )
                        nc.scalar.copy(
                            out_sb[0:mw, nh2 * 512:(nh2 + 1) * 512], op[0:mw, :]
                        )
                    ro = b * TB + h * TBH + t0
                    nc.sync.dma_start(out[ro:ro + mw, :], out_sb[0:mw, :])

def build_graph(cfg):
    import concourse.mybir as mybir
    import concourse.tile as tile
    from concourse import bacc

    dt = mybir.dt
    nc = bacc.Bacc("TRN2", target_bir_lowering=False, debug=False,
                   num_devices=NCORES)
    TLOC = cfg["TLOC"]
    ins = {
        "x": nc.dram_tensor("x", [TLOC, DIM], dt.float32, kind="ExternalInput").ap(),
        "g": nc.dram_tensor("g", [DIM], dt.float32, kind="ExternalInput").ap(),
        "wq": nc.dram_tensor("wq", [DIM, DHEAD], dt.float32, kind="ExternalInput").ap(),
        "wk": nc.dram_tensor("wk", [DIM, DHEAD], dt.float32, kind="ExternalInput").ap(),
        "wv": nc.dram_tensor("wv", [DIM, DHEAD], dt.float32, kind="ExternalInput").ap(),
        "wo": nc.dram_tensor("wo", [INNER, DIM], dt.float32, kind="ExternalInput").ap(),
        "bsel": nc.dram_tensor("bsel", [P, 2], dt.float32, kind="ExternalInput").ap(),
    }
    outs = {
        "out": nc.dram_tensor("out", [TLOC, DIM], dt.float32,
                              kind="ExternalOutput").ap(),
    }
    with tile.TileContext(nc) as tc:
        build_body(tc, outs, ins, cfg)
    nc.compile()
    return nc


def make_in_maps(x, g, wq, wkv, wo, cfg):
    """Shard full inputs into per-core input maps."""
    T, TLOC = cfg["T"], cfg["TLOC"]
    nseq = cfg["nseq"]
    x2 = np.ascontiguousarray(np.asarray(x, np.float32).reshape(T, DIM))
    g_ = np.ascontiguousarray(np.asarray(g, np.float32))
    wq_ = np.asarray(wq, np.float32)
    wkv_ = np.asarray(wkv, np.float32)
    wo_ = np.ascontiguousarray(np.asarray(wo, np.float32))
    wk_all = wkv_[:, :INNER]
    wv_all = wkv_[:, INNER:]
    in_maps = []
    for c in range(NCORES):
        h = c
        bsel = np.zeros((P, 2), np.float32)
        bsel[:, (c * TLOC) // nseq] = 1.0
        in_maps.append({
            "x": np.ascontiguousarray(x2[c * TLOC:(c + 1) * TLOC]),
            "g": g_,
            "wq": np.ascontiguousarray(wq_[:, h * DHEAD:(h + 1) * DHEAD]),
            "wk": np.ascontiguousarray(wk_all[:, h * DHEAD:(h + 1) * DHEAD]),
            "wv": np.ascontiguousarray(wv_all[:, h * DHEAD:(h + 1) * DHEAD]),
            "wo": wo_,
            "bsel": bsel,
        })
    return in_maps


def assemble_out(core_outs, cfg):
    """Core c's rows: per (batch, half): tokens b*nseq + h*nseq/NH + c*TBH."""
    T, TB = cfg["T"], cfg["TB"]
    nseq = cfg["nseq"]
    NH = 2 if cfg["IB"] >= 2 else 1
    TBH = TB // NH
    full = np.empty((T, DIM), np.float32)
    for c in range(NCORES):
        o = core_outs[c]
        for b in range(B):
            for h in range(NH):
                src = b * TB + h * TBH
                dst = b * nseq + h * (nseq // NH) + c * TBH
                full[dst:dst + TBH] = o[src:src + TBH]
    return full


_cache = {}


def _get_graph():
    if "nc" not in _cache:
        _cache["nc"] = build_graph(_cfg())
    return _cache["nc"]


def run_on_hw(in_maps, trace=False, **kw):
    from concourse.bass_utils import run_bass_kernel_spmd
    nc = _get_graph()
    return run_bass_kernel_spmd(
        nc, in_maps, core_ids=list(range(NCORES)), trace=trace, **kw
    )


def kernel(x, g, wq, wkv, wo):
    cfg = _cfg()
    in_maps = make_in_maps(x, g, wq, wkv, wo, cfg)
    res = run_on_hw(in_maps)
    core_outs = [np.asarray(res.results[c]["out"], np.float32)
                 for c in range(NCORES)]
    return assemble_out(core_outs, cfg).reshape(B, NSEQ, DIM)
